# revision 1
# baseline (speedup 1.0000x reference)
"""GPT forward (embed + 1 causal attention block + LM head) on 8 TRN2 cores.

Sharding: every core runs QKV+attention for all heads (redundant, small);
the dominant LM head (V=50257 x C=1024) is vocab-sharded: core r computes
logits for vocab rows [r*6400, (r+1)*6400) (padded to 51200).

Device layout conventions (matching concourse tile_matmul):
  A [R, Cc] matrix lives in DRAM as [128, R/128, Cc] with row r = k*128+p.
  matmul computes psum[M,N] = lhsT[K,M].T @ rhs[K,N]  (K = partitions).
"""

import numpy as np

import concourse.bass as bass
import concourse.mybir as mybir
import concourse.tile as tile
from concourse.bass_utils import run_bass_kernel_spmd
from concourse.kernels.tile_matmul import matmul_tile_kernel
from concourse.masks import make_identity

B, T, C, H, HD, V = 2, 1024, 1024, 16, 64, 50257
BT = B * T
NCORES = 8
VS = 6400               # per-core padded vocab shard
VPAD = VS * NCORES      # 51200
P = 128
KT = C // P             # 8 k-subtiles of the C contraction
NPAIR = H // 2          # 8 head pairs (2 heads = 128 output dims)
NKV = T // P            # 8 kv tiles per batch
QS = 512                # q strip width for score matmuls
F32 = mybir.dt.float32

_built = {}


def _split_multiwait(nc, max_waits=1):
    """This container's walrus rejects >1 sync wait per instruction; move
    extra waits onto inserted single-wait NoOps on the same engine."""
    n = 0
    for fn in nc.m.functions:
        for blk in fn.blocks:
            new_insts = []
            for ins in blk.instructions:
                si = getattr(ins, "sync_info", None)
                ow = list(si.on_wait) if (si is not None and si.on_wait) else []
                if len(ow) > max_waits:
                    extra, keep = ow[:-max_waits], ow[-max_waits:]
                    for k, w in enumerate(extra):
                        n += 1
                        new_insts.append(mybir.InstNoOp(
                            name=f"{ins.name}-ws{k}",
                            engine=ins.engine,
                            ins=[], outs=[],
                            sync_info=mybir.SyncInfo(on_wait=[w], on_update=[]),
                        ))
                    si.on_wait = keep
                new_insts.append(ins)
            blk.instructions = new_insts
    return n


def _build_program():
    if "nc" in _built:
        return _built["nc"]
    nc = bass.Bass()

    xT = nc.declare_dram_parameter("xT", [P, KT, BT], F32, isOutput=False)
    # [proj(q/k/v), pair, p, k, m] ; lhsT tile for a pair = [:, :, p, k, m]
    wqkv = nc.declare_dram_parameter("wqkv", [3, NPAIR, P, KT, P], F32, isOutput=False)
    wlmT = nc.declare_dram_parameter("wlmT", [P, KT, VS], F32, isOutput=False)
    masks = nc.declare_dram_parameter("masks", [P, 4, QS], F32, isOutput=False)
    logitsT = nc.declare_dram_parameter("logitsT", [P, VS // P, BT], F32, isOutput=True)

    oT_d = nc.dram_tensor("oT_d", [P, KT, BT], F32)

    with tile.TileContext(nc) as tc:
        _attention_phase(nc, tc, xT, wqkv, masks, oT_d)
        matmul_tile_kernel(
            tc,
            wlmT[:],
            oT_d[:],
            logitsT[:],
        )

    _split_multiwait(nc)
    _built["nc"] = nc
    return nc


def _attention_phase(nc, tc, xT, wqkv, masks, oT_d):
    from contextlib import ExitStack

    with ExitStack() as ctx:
        xpool = ctx.enter_context(tc.tile_pool(name="xpool", bufs=1))
        constp = ctx.enter_context(tc.tile_pool(name="constp", bufs=1))
        wpool = ctx.enter_context(tc.tile_pool(name="wpool", bufs=2))
        qkpool = ctx.enter_context(tc.tile_pool(name="qkpool", bufs=2))
        vpool = ctx.enter_context(tc.tile_pool(name="vpool", bufs=2))
        epool = ctx.enter_context(tc.tile_pool(name="epool", bufs=9))
        opool = ctx.enter_context(tc.tile_pool(name="opool", bufs=3))
        ps_big = ctx.enter_context(tc.tile_pool(name="ps_big", bufs=3, space="PSUM"))
        ps_o = ctx.enter_context(tc.tile_pool(name="ps_o", bufs=2, space="PSUM"))
        ps_t = ctx.enter_context(tc.tile_pool(name="ps_t", bufs=2, space="PSUM"))

        xT_s = xpool.tile([P, KT, BT], F32)
        nc.sync.dma_start(out=xT_s[:], in_=xT[:])
        mask_s = constp.tile([P, 4, QS], F32)
        nc.sync.dma_start(out=mask_s[:], in_=masks[:])
        ident = constp.tile([P, P], F32)
        make_identity(nc, ident[:])

        for j in range(NPAIR):
            wq_s = wpool.tile([P, KT, P], F32, tag="wq")
            wk_s = wpool.tile([P, KT, P], F32, tag="wk")
            wv_s = wpool.tile([P, KT, P], F32, tag="wv")
            nc.sync.dma_start(out=wq_s[:], in_=wqkv[0, j])
            nc.sync.dma_start(out=wk_s[:], in_=wqkv[1, j])
            nc.sync.dma_start(out=wv_s[:], in_=wqkv[2, j])

            # qT/kT for the pair: [128 (pair dims), BT]
            qT_s = qkpool.tile([P, BT], F32, tag="qT")
            kT_s = qkpool.tile([P, BT], F32, tag="kT")
            for w_s, dst in ((wq_s, qT_s), (wk_s, kT_s)):
                for qi in range(BT // QS):
                    pp = ps_big.tile([P, QS], F32, tag="pbig")
                    for k in range(KT):
                        nc.tensor.matmul(
                            pp[:],
                            w_s[:, k, :],
                            xT_s[:, k, qi * QS:(qi + 1) * QS],
                            start=(k == 0),
                            stop=(k == KT - 1),
                        )
                    nc.scalar.copy(dst[:, qi * QS:(qi + 1) * QS], pp[:])

            # v for the pair, augmented with ones columns at both ends:
            # [128 kv, 16 tiles, 130] ; col0 = ones, 1..128 = pair dims, 129 = ones
            v_s = vpool.tile([P, BT // P, 130], F32, tag="v")
            nc.vector.memset(v_s[:, :, 0:1], 1.0)
            nc.vector.memset(v_s[:, :, 129:130], 1.0)
            for bt in range(BT // P):
                pv = ps_big.tile([P, QS], F32, tag="pbig")
                for k in range(KT):
                    nc.tensor.matmul(
                        pv[:, :P],
                        xT_s[:, k, bt * P:(bt + 1) * P],
                        wv_s[:, k, :],
                        start=(k == 0),
                        stop=(k == KT - 1),
                    )
                nc.scalar.copy(v_s[:, bt, 1:129], pv[:, :P])

            for b in range(B):
                for e in range(2):
                    _head_attention(
                        nc, tc, b, e, j, qT_s, kT_s, v_s, mask_s, ident,
                        epool, opool, ps_big, ps_o, ps_t, oT_d,
                    )


def _head_attention(nc, tc, b, e, j, qT_s, kT_s, v_s, mask_s, ident,
                    epool, opool, ps_big, ps_o, ps_t, oT_d):
    """One (batch, head) causal attention; writes oT slab rows to DRAM."""
    prow = slice(e * HD, (e + 1) * HD)   # this head's 64 dims in the pair tile
    q0 = b * T
    for qi in range(T // QS):
        exps = []
        for nj in range(4 * qi + 4):
            sp = ps_big.tile([P, QS], F32, tag="pbig")
            nc.tensor.matmul(
                sp[:],
                kT_s[prow, q0 + nj * P: q0 + (nj + 1) * P],
                qT_s[prow, q0 + qi * QS: q0 + (qi + 1) * QS],
                start=True,
                stop=True,
            )
            ex = epool.tile([P, QS], F32, tag="exp")
            nc.scalar.activation(ex[:], sp[:], mybir.ActivationFunctionType.Exp)
            t = nj - 4 * qi
            if t >= 0:
                nc.vector.tensor_mul(ex[:], ex[:], mask_s[:, t, :])
            exps.append(ex)

        for qj in range(QS // P):
            m = 4 * qi + qj            # global q tile within the batch
            po = ps_o.tile([P, 66], F32, tag="po")
            voff = 0 if e == 0 else 65
            for nj in range(m + 1):
                nc.tensor.matmul(
                    po[:, :65],
                    exps[nj][:, qj * P:(qj + 1) * P],
                    v_s[:, b * NKV + nj, voff:voff + 65],
                    start=(nj == 0),
                    stop=(nj == m),
                )
            dcol = 0 if e == 0 else 64
            dslice = slice(1, 65) if e == 0 else slice(0, 64)
            rec = opool.tile([P, 1], F32, tag="rec")
            nc.vector.reciprocal(rec[:], po[:, dcol:dcol + 1])
            onorm = opool.tile([P, HD], F32, tag="onorm")
            nc.scalar.activation(
                onorm[:], po[:, dslice],
                mybir.ActivationFunctionType.Copy, scale=rec[:],
            )
            tp = ps_t.tile([HD, P], F32, tag="pt")
            nc.tensor.transpose(tp[:], onorm[:], ident[:])
            oslab = opool.tile([HD, P], F32, tag="oslab")
            nc.vector.tensor_copy(oslab[:], tp[:])
            nc.sync.dma_start(
                out=oT_d[e * HD:(e + 1) * HD, j, q0 + m * P: q0 + (m + 1) * P],
                in_=oslab[:],
            )


def _host_prep(idx, tok_emb, pos_emb, Wq, Wk, Wv, W_lm):
    x = tok_emb[idx.reshape(-1)].astype(np.float32) + np.tile(
        pos_emb[:T].astype(np.float32), (B, 1)
    )  # [BT, C]
    xT_in = np.ascontiguousarray(
        x.T.reshape(KT, P, BT).transpose(1, 0, 2)
    )  # [P, KT, BT]

    def pack_w(W):
        # W [H, C, HD] -> [NPAIR, P, KT, 128] with [j,p,k,e*64+d] = W[2j+e, k*128+p, d]
        return np.ascontiguousarray(
            W.reshape(NPAIR, 2, KT, P, HD).transpose(0, 3, 2, 1, 4).reshape(
                NPAIR, P, KT, P
            )
        )

    wqkv = np.stack([
        pack_w(Wq.astype(np.float32) * (C ** -0.5)),
        pack_w(Wk.astype(np.float32)),
        pack_w(Wv.astype(np.float32)),
    ])  # [3, NPAIR, P, KT, P]

    pm = np.arange(P)[:, None]
    fm = np.arange(QS)[None, :]
    masks = np.stack(
        [(fm >= t * P + pm).astype(np.float32) for t in range(4)], axis=1
    )  # [P, 4, QS]

    W_lm_pad = np.zeros((VPAD, C), np.float32)
    W_lm_pad[:V] = W_lm.astype(np.float32)
    wlmT_shards = []
    for r in range(NCORES):
        sh = W_lm_pad[r * VS:(r + 1) * VS].T  # [C, VS]
        wlmT_shards.append(
            np.ascontiguousarray(sh.reshape(KT, P, VS).transpose(1, 0, 2))
        )
    return xT_in, wqkv, masks, wlmT_shards


def kernel(idx, tok_emb, pos_emb, Wq, Wk, Wv, W_lm, b_lm, _trace=False):
    idx = np.asarray(idx)
    xT_in, wqkv, masks, wlmT_shards = _host_prep(
        np.asarray(idx), np.asarray(tok_emb), np.asarray(pos_emb),
        np.asarray(Wq), np.asarray(Wk), np.asarray(Wv), np.asarray(W_lm),
    )
    nc = _build_program()
    in_maps = [
        {"xT": xT_in, "wqkv": wqkv, "wlmT": wlmT_shards[r], "masks": masks}
        for r in range(NCORES)
    ]
    res = run_bass_kernel_spmd(nc, in_maps, list(range(NCORES)), trace=_trace)
    parts = []
    for r in range(NCORES):
        lt = res.results[r]["logitsT"]  # [P, VS//P, BT]
        parts.append(np.asarray(lt).transpose(1, 0, 2).reshape(VS, BT))
    full = np.concatenate(parts, axis=0)[:V]          # [V, BT]
    logits = np.ascontiguousarray(full.T).reshape(B, T, V)
    b_lm = np.asarray(b_lm, dtype=np.float32)
    if np.any(b_lm):
        logits = logits + b_lm
    if _trace:
        kernel._last_exec_time_ns = res.exec_time_ns
        kernel._last_profile_json = res.profile_json
    return logits.astype(np.float32)



# revision 12
# speedup vs baseline: 1.3461x; 1.3461x over previous
"""GPT forward (embed + 1 causal attention block + LM head) on 8 TRN2 cores.

Sharding (all fp32, numerically equivalent to the reference):
  - Attention is tensor-parallel over the 16 heads: core r computes heads
    {2r, 2r+1} (one "pair" = 128 output dims) for both batches, then an
    AllGather over the 8 cores reconstructs the full attention output
    x_outT [C=1024, B*T] on every core.
  - The LM head (V=50257 x C=1024, the dominant cost) is vocab-sharded:
    core r computes logits for vocab rows [r*6400, (r+1)*6400) (padded to
    51200) with a hand-tiled matmul loop (full-size 128x128x512 matmuls,
    multi-bank PSUM, psum evictions spread over scalar/vector/gpsimd).

Device layout conventions:
  A [R, Cc] matrix lives in DRAM as [128, R/128, Cc] with row r = k*128+p.
  matmul computes psum[M,N] = lhsT[K,M].T @ rhs[K,N]  (K = partitions).
"""

import numpy as np

import concourse.bass as bass
import concourse.mybir as mybir
import concourse.tile as tile
from concourse.bass_utils import run_bass_kernel_spmd

B, T, C, H, HD, V = 2, 1024, 1024, 16, 64, 50257
BT = B * T
NCORES = 8
VS = 6400               # per-core padded vocab shard
VPAD = VS * NCORES      # 51200
P = 128
KT = C // P             # 8 k-subtiles of the C contraction
NKV = T // P            # 8 kv tiles per batch
QS = 512                # q strip width for score matmuls
MT = VS // P            # 50 vocab m-tiles per core
NT = BT // QS           # 4 n-tiles for the LM head
F32 = mybir.dt.float32

_built = {}


def _split_multiwait(nc, max_waits=1):
    """This container's walrus rejects >1 sync wait per instruction; move
    extra waits onto inserted single-wait NoOps on the same engine."""
    n = 0
    for fn in nc.m.functions:
        for blk in fn.blocks:
            new_insts = []
            for ins in blk.instructions:
                si = getattr(ins, "sync_info", None)
                ow = list(si.on_wait) if (si is not None and si.on_wait) else []
                if len(ow) > max_waits:
                    extra, keep = ow[:-max_waits], ow[-max_waits:]
                    for k, w in enumerate(extra):
                        n += 1
                        new_insts.append(mybir.InstNoOp(
                            name=f"{ins.name}-ws{k}",
                            engine=ins.engine,
                            ins=[], outs=[],
                            sync_info=mybir.SyncInfo(on_wait=[w], on_update=[]),
                        ))
                    si.on_wait = keep
                new_insts.append(ins)
            blk.instructions = new_insts
    return n


def _build_program():
    if "nc" in _built:
        return _built["nc"]
    nc = bass.Bass(num_devices=NCORES)

    xT = nc.declare_dram_parameter("xT", [P, KT, BT], F32, isOutput=False)
    # this core's head pair: [proj(q/k/v), p, k, m]
    wqkv = nc.declare_dram_parameter("wqkv", [3, P, KT, P], F32, isOutput=False)
    # m-tile-major so each [P, KT, 128] weight-tile DMA is contiguous
    wlmT = nc.declare_dram_parameter("wlmT", [MT, P, KT, P], F32, isOutput=False)
    masks = nc.declare_dram_parameter("masks", [P, 4, QS], F32, isOutput=False)
    logitsT = nc.declare_dram_parameter("logitsT", [P, MT, BT], F32, isOutput=True)

    # collective bounce buffers (must be Internal DRAM, Shared output)
    ag_in = nc.dram_tensor("ag_in", [P, BT], F32)
    ag_out = nc.dram_tensor("ag_out", [NCORES, P, BT], F32, addr_space="Shared")

    with tile.TileContext(nc) as tc:
        _attention_phase(nc, tc, xT, wqkv, masks, ag_in)
        nc.gpsimd.collective_compute(
            "AllGather",
            mybir.AluOpType.bypass,
            replica_groups=[list(range(NCORES))],
            ins=[ag_in[:].opt()],
            outs=[ag_out[:].opt()],
        )
        _lm_head_phase(nc, tc, wlmT, ag_out, logitsT)

    _split_multiwait(nc)
    _built["nc"] = nc
    return nc


def _attention_phase(nc, tc, xT, wqkv, masks, ag_in):
    """Causal attention for this core's head pair; writes the normalized
    x_outT slab [128 pair dims, BT] to ag_in."""
    from contextlib import ExitStack

    with ExitStack() as ctx:
        xpool = ctx.enter_context(tc.tile_pool(name="xpool", bufs=1))
        constp = ctx.enter_context(tc.tile_pool(name="constp", bufs=1))
        wpool = ctx.enter_context(tc.tile_pool(name="wpool", bufs=1))
        qkpool = ctx.enter_context(tc.tile_pool(name="qkpool", bufs=1))
        vpool = ctx.enter_context(tc.tile_pool(name="vpool", bufs=1))
        epool = ctx.enter_context(tc.tile_pool(name="epool", bufs=9))
        opool = ctx.enter_context(tc.tile_pool(name="opool", bufs=4))
        ps_big = ctx.enter_context(tc.tile_pool(name="ps_big", bufs=3, space="PSUM"))
        ps_o = ctx.enter_context(tc.tile_pool(name="ps_o", bufs=2, space="PSUM"))
        ps_b = ctx.enter_context(tc.tile_pool(name="ps_b", bufs=2, space="PSUM"))

        xT_s = xpool.tile([P, KT, BT], F32)
        nc.sync.dma_start(out=xT_s[:], in_=xT[:])
        mask_s = constp.tile([P, 4, QS], F32)
        nc.sync.dma_start(out=mask_s[:], in_=masks[:])
        ones_s = constp.tile([1, P], F32)
        nc.vector.memset(ones_s[:], 1.0)

        wq_s = wpool.tile([P, KT, P], F32, tag="wq")
        wk_s = wpool.tile([P, KT, P], F32, tag="wk")
        wv_s = wpool.tile([P, KT, P], F32, tag="wv")
        nc.sync.dma_start(out=wq_s[:], in_=wqkv[0])
        nc.sync.dma_start(out=wk_s[:], in_=wqkv[1])
        nc.sync.dma_start(out=wv_s[:], in_=wqkv[2])

        # qT/kT for the pair: [128 (pair dims), BT]
        qT_s = qkpool.tile([P, BT], F32, tag="qT")
        kT_s = qkpool.tile([P, BT], F32, tag="kT")
        for w_s, dst, ev in (
            (wq_s, qT_s, lambda o, i: nc.scalar.copy(o, i)),
            (wk_s, kT_s, lambda o, i: nc.vector.tensor_copy(o, i)),
        ):
            for qi in range(BT // QS):
                pp = ps_big.tile([P, QS], F32, tag="pbig")
                for k in range(KT):
                    nc.tensor.matmul(
                        pp[:],
                        w_s[:, k, :],
                        xT_s[:, k, qi * QS:(qi + 1) * QS],
                        start=(k == 0),
                        stop=(k == KT - 1),
                    )
                ev(dst[:, qi * QS:(qi + 1) * QS], pp[:])

        # v for the pair, augmented with a ones column after each head:
        # [128 kv, 16 tiles, 130] ; cols 0..63 = head-0 dims, col 64 = ones,
        # cols 65..128 = head-1 dims, col 129 = ones. Each head's AV lhsT
        # window [65e : 65e+65] = [64 dims, ones] -> psum rows 0..63 = out
        # dims, row 64 = softmax denominator (aligned partition start).
        v_s = vpool.tile([P, BT // P, 130], F32, tag="v")
        nc.vector.memset(v_s[:, :, 64:65], 1.0)
        nc.vector.memset(v_s[:, :, 129:130], 1.0)
        for bt in range(BT // P):
            pv = ps_big.tile([P, QS], F32, tag="pbig")
            for k in range(KT):
                nc.tensor.matmul(
                    pv[:, :P],
                    xT_s[:, k, bt * P:(bt + 1) * P],
                    wv_s[:, k, :],
                    start=(k == 0),
                    stop=(k == KT - 1),
                )
            if bt % 2:
                nc.scalar.copy(v_s[:, bt, 0:64], pv[:, 0:64])
                nc.scalar.copy(v_s[:, bt, 65:129], pv[:, 64:128])
            else:
                nc.vector.tensor_copy(v_s[:, bt, 0:64], pv[:, 0:64])
                nc.vector.tensor_copy(v_s[:, bt, 65:129], pv[:, 64:128])

        # attention per (batch, head-in-pair, q strip)
        for b in range(B):
            for e in range(2):
                prow = slice(e * HD, (e + 1) * HD)
                voff = 65 * e
                q0 = b * T
                for qi in range(T // QS):
                    nkv = 4 * qi + 4
                    exps = []
                    for nj in range(nkv):
                        sp = ps_big.tile([P, QS], F32, tag="pbig")
                        nc.tensor.matmul(
                            sp[:],
                            kT_s[prow, q0 + nj * P: q0 + (nj + 1) * P],
                            qT_s[prow, q0 + qi * QS: q0 + (qi + 1) * QS],
                            start=True,
                            stop=True,
                        )
                        ex = epool.tile([P, QS], F32, tag="exp")
                        nc.scalar.activation(
                            ex[:], sp[:], mybir.ActivationFunctionType.Exp)
                        t = nj - 4 * qi
                        if t >= 0:
                            nc.vector.tensor_mul(ex[:], ex[:], mask_s[:, t, :])
                        exps.append(ex)

                    po = ps_o.tile([65, QS], F32, tag="po")
                    for nj in range(nkv):
                        nc.tensor.matmul(
                            po[:],
                            v_s[:, b * NKV + nj, voff:voff + 65],
                            exps[nj][:],
                            start=(nj == 0),
                            stop=(nj == nkv - 1),
                        )
                    # normalize along q (free dim): reciprocal of the denom
                    # row, broadcast to 64 partitions via a K=1 matmul
                    rec = opool.tile([1, QS], F32, tag="rec")
                    nc.vector.reciprocal(rec[:], po[64:65, :])
                    pb = ps_b.tile([64, QS], F32, tag="pb")
                    nc.tensor.matmul(
                        pb[:], ones_s[:, :64], rec[:], start=True, stop=True)
                    rec_b = opool.tile([64, QS], F32, tag="recb")
                    nc.scalar.copy(rec_b[:], pb[:])
                    xo = opool.tile([64, QS], F32, tag="xo")
                    nc.vector.tensor_mul(xo[:], po[0:64, :], rec_b[:])
                    nc.sync.dma_start(
                        out=ag_in[e * HD:(e + 1) * HD,
                                  q0 + qi * QS: q0 + (qi + 1) * QS],
                        in_=xo[:],
                    )


def _lm_head_phase(nc, tc, wlmT, ag_out, logitsT):
    """logits shard [VS, BT] = W_shard[C, VS].T @ x_outT[C, BT]."""
    from contextlib import ExitStack

    with ExitStack() as ctx:
        xop = ctx.enter_context(tc.tile_pool(name="xop", bufs=1))
        wp = ctx.enter_context(tc.tile_pool(name="wp", bufs=3))
        outp = ctx.enter_context(tc.tile_pool(name="outp", bufs=6))
        psp = ctx.enter_context(tc.tile_pool(name="psp", bufs=8, space="PSUM"))

        xout_s = xop.tile([P, KT, BT], F32)
        for k in range(KT):
            nc.sync.dma_start(out=xout_s[:, k, :], in_=ag_out[k])

        for m in range(MT):
            w_s = wp.tile([P, KT, P], F32, tag="w")
            nc.sync.dma_start(out=w_s[:], in_=wlmT[m])
            for n in range(NT):
                ps = psp.tile([P, QS], F32, tag="ps")
                for k in range(KT):
                    nc.tensor.matmul(
                        ps[:],
                        w_s[:, k, :],
                        xout_s[:, k, n * QS:(n + 1) * QS],
                        start=(k == 0),
                        stop=(k == KT - 1),
                    )
                o_s = outp.tile([P, QS], F32, tag="o")
                if (m * NT + n) % 2:
                    nc.scalar.copy(o_s[:], ps[:])
                else:
                    nc.vector.tensor_copy(o_s[:], ps[:])
                nc.sync.dma_start(
                    out=logitsT[:, m, n * QS:(n + 1) * QS], in_=o_s[:])


def _host_prep(idx, tok_emb, pos_emb, Wq, Wk, Wv, W_lm):
    x = tok_emb[idx.reshape(-1)].astype(np.float32) + np.tile(
        pos_emb[:T].astype(np.float32), (B, 1)
    )  # [BT, C]
    xT_in = np.ascontiguousarray(
        x.T.reshape(KT, P, BT).transpose(1, 0, 2)
    )  # [P, KT, BT]

    NPAIR = H // 2

    def pack_w(W):
        # W [H, C, HD] -> [NPAIR, P, KT, 128] with [j,p,k,e*64+d] = W[2j+e, k*128+p, d]
        return np.ascontiguousarray(
            W.reshape(NPAIR, 2, KT, P, HD).transpose(0, 3, 2, 1, 4).reshape(
                NPAIR, P, KT, P
            )
        )

    wqkv = np.stack([
        pack_w(Wq.astype(np.float32) * (C ** -0.5)),
        pack_w(Wk.astype(np.float32)),
        pack_w(Wv.astype(np.float32)),
    ])  # [3, NPAIR, P, KT, P]

    pm = np.arange(P)[:, None]
    fm = np.arange(QS)[None, :]
    masks = np.stack(
        [(fm >= t * P + pm).astype(np.float32) for t in range(4)], axis=1
    )  # [P, 4, QS]

    W_lm_pad = np.zeros((VPAD, C), np.float32)
    W_lm_pad[:V] = W_lm.astype(np.float32)
    wlmT_shards = []
    for r in range(NCORES):
        sh = W_lm_pad[r * VS:(r + 1) * VS]  # [VS, C]
        # [MT, P, KT, P] with [m, p, k, j] = W[m*128 + j, k*128 + p]
        wlmT_shards.append(np.ascontiguousarray(
            sh.reshape(MT, P, KT, P).transpose(0, 3, 2, 1)
        ))
    return xT_in, wqkv, masks, wlmT_shards


def kernel(idx, tok_emb, pos_emb, Wq, Wk, Wv, W_lm, b_lm, _trace=False):
    idx = np.asarray(idx)
    xT_in, wqkv, masks, wlmT_shards = _host_prep(
        np.asarray(idx), np.asarray(tok_emb), np.asarray(pos_emb),
        np.asarray(Wq), np.asarray(Wk), np.asarray(Wv), np.asarray(W_lm),
    )
    nc = _build_program()
    in_maps = [
        {
            "xT": xT_in,
            "wqkv": np.ascontiguousarray(wqkv[:, r]),
            "wlmT": wlmT_shards[r],
            "masks": masks,
        }
        for r in range(NCORES)
    ]
    res = run_bass_kernel_spmd(nc, in_maps, list(range(NCORES)), trace=_trace)
    parts = []
    for r in range(NCORES):
        lt = res.results[r]["logitsT"]  # [P, MT, BT]
        parts.append(np.asarray(lt).transpose(1, 0, 2).reshape(VS, BT))
    full = np.concatenate(parts, axis=0)[:V]          # [V, BT]
    logits = np.ascontiguousarray(full.T).reshape(B, T, V)
    b_lm = np.asarray(b_lm, dtype=np.float32)
    if np.any(b_lm):
        logits = logits + b_lm
    if _trace:
        kernel._last_exec_time_ns = res.exec_time_ns
        kernel._last_profile_json = res.profile_json
    return logits.astype(np.float32)


# revision 17
# speedup vs baseline: 4.1634x; 3.0929x over previous
"""GPT forward (embed + 1 causal attention block + LM head) on 8 TRN2 cores.

Sharding:
  - Attention is tensor-parallel over the 16 heads: core r computes heads
    {2r, 2r+1} (one "pair" = 128 output dims) for both batches, then an
    AllGather over the 8 cores reconstructs the full attention output
    x_outT [C=1024, B*T] on every core.
  - The LM head (V=50257 x C=1024, the dominant cost) is vocab-sharded:
    core r computes logits for vocab rows [r*6400, (r+1)*6400) (padded to
    51200) with a hand-tiled matmul loop (128x128x512 matmuls, multi-bank
    PSUM, psum evictions split over scalar/vector).

Precision: data stays fp32 end-to-end in memory; matmuls run with the
operands bitcast to float32r (single-pass PE mode, ~3x the fp32 rate,
~1.5e-4 relative error vs ~2e-3 for bf16). PSUM accumulates in fp32.
Logits are stored to DRAM as fp16 (proportional rounding only) to halve
the 52MB output-DMA stream; the host widens them back to fp32.

Device layout conventions:
  A [R, Cc] matrix lives in DRAM as [128, R/128, Cc] with row r = k*128+p.
  matmul computes psum[M,N] = lhsT[K,M].T @ rhs[K,N]  (K = partitions).
"""

import numpy as np

import concourse.bass as bass
import concourse.mybir as mybir
import concourse.tile as tile
from concourse.bass_utils import run_bass_kernel_spmd

B, T, C, H, HD, V = 2, 1024, 1024, 16, 64, 50257
BT = B * T
NCORES = 8
VS = 6400               # per-core padded vocab shard
VPAD = VS * NCORES      # 51200
P = 128
KT = C // P             # 8 k-subtiles of the C contraction
NKV = T // P            # 8 kv tiles per batch
QS = 512                # q strip width for score matmuls
MT = VS // P            # 50 vocab m-tiles per core
NT = BT // QS           # 4 n-tiles for the LM head
F32 = mybir.dt.float32
F32R = mybir.dt.float32r
F16 = mybir.dt.float16
WPREF = 3               # LM weight tiles prefetched during attention

_built = {}


def _mm(nc, out, lhsT, rhs, **kw):
    """matmul with operands reinterpreted as float32r (1-pass PE mode)."""
    nc.tensor.matmul(out, lhsT.bitcast(F32R), rhs.bitcast(F32R), **kw)


def _split_multiwait(nc, max_waits=1):
    """This container's walrus rejects >1 sync wait per instruction; move
    extra waits onto inserted single-wait NoOps on the same engine."""
    n = 0
    for fn in nc.m.functions:
        for blk in fn.blocks:
            new_insts = []
            for ins in blk.instructions:
                si = getattr(ins, "sync_info", None)
                ow = list(si.on_wait) if (si is not None and si.on_wait) else []
                if len(ow) > max_waits:
                    extra, keep = ow[:-max_waits], ow[-max_waits:]
                    for k, w in enumerate(extra):
                        n += 1
                        new_insts.append(mybir.InstNoOp(
                            name=f"{ins.name}-ws{k}",
                            engine=ins.engine,
                            ins=[], outs=[],
                            sync_info=mybir.SyncInfo(on_wait=[w], on_update=[]),
                        ))
                    si.on_wait = keep
                new_insts.append(ins)
            blk.instructions = new_insts
    return n


def _build_program():
    if "nc" in _built:
        return _built["nc"]
    nc = bass.Bass(num_devices=NCORES)

    xT = nc.declare_dram_parameter("xT", [P, KT, BT], F32, isOutput=False)
    # this core's head pair: [proj(q/k/v), p, k, m]
    wqkv = nc.declare_dram_parameter("wqkv", [3, P, KT, P], F32, isOutput=False)
    # m-tile-major so each [P, KT, 128] weight-tile DMA is contiguous
    wlmT = nc.declare_dram_parameter("wlmT", [MT, P, KT, P], F32, isOutput=False)
    masks = nc.declare_dram_parameter("masks", [P, 4, QS], F32, isOutput=False)
    logitsT = nc.declare_dram_parameter("logitsT", [P, MT, BT], F16, isOutput=True)

    # collective bounce buffers (must be Internal DRAM, Shared output)
    ag_in = nc.dram_tensor("ag_in", [P, BT], F32)
    ag_out = nc.dram_tensor("ag_out", [NCORES, P, BT], F32, addr_space="Shared")

    with tile.TileContext(nc) as tc:
        from contextlib import ExitStack
        with ExitStack() as octx:
            # weight pool opens before attention so the first WPREF LM
            # weight-tile DMAs run during the attention phase
            wp = octx.enter_context(tc.tile_pool(name="wp", bufs=WPREF + 1))
            w_tiles = []
            for m in range(WPREF):
                w_s = wp.tile([P, KT, P], F32R, tag="w")
                nc.sync.dma_start(out=w_s[:], in_=wlmT[m].bitcast(F32R))
                w_tiles.append(w_s)

            _attention_phase(nc, tc, xT, wqkv, masks, ag_in)
            nc.gpsimd.collective_compute(
                "AllGather",
                mybir.AluOpType.bypass,
                replica_groups=[list(range(NCORES))],
                ins=[ag_in[:].opt()],
                outs=[ag_out[:].opt()],
            )
            _lm_head_phase(nc, tc, wp, w_tiles, wlmT, ag_out, logitsT)

    _split_multiwait(nc)
    _built["nc"] = nc
    return nc


def _attention_phase(nc, tc, xT, wqkv, masks, ag_in):
    """Causal attention for this core's head pair; writes the normalized
    x_outT slab [128 pair dims, BT] to ag_in."""
    from contextlib import ExitStack

    with ExitStack() as ctx:
        xpool = ctx.enter_context(tc.tile_pool(name="xpool", bufs=1))
        constp = ctx.enter_context(tc.tile_pool(name="constp", bufs=1))
        wpool = ctx.enter_context(tc.tile_pool(name="wpool", bufs=1))
        qkpool = ctx.enter_context(tc.tile_pool(name="qkpool", bufs=1))
        vpool = ctx.enter_context(tc.tile_pool(name="vpool", bufs=1))
        epool = ctx.enter_context(tc.tile_pool(name="epool", bufs=9))
        opool = ctx.enter_context(tc.tile_pool(name="opool", bufs=4))
        ps_big = ctx.enter_context(tc.tile_pool(name="ps_big", bufs=3, space="PSUM"))
        ps_o = ctx.enter_context(tc.tile_pool(name="ps_o", bufs=2, space="PSUM"))
        ps_b = ctx.enter_context(tc.tile_pool(name="ps_b", bufs=2, space="PSUM"))

        xT_s = xpool.tile([P, KT, BT], F32R)
        nc.sync.dma_start(out=xT_s[:], in_=xT[:].bitcast(F32R))
        mask_s = constp.tile([P, 4, QS], F32)
        nc.sync.dma_start(out=mask_s[:], in_=masks[:])
        ones_f = constp.tile([1, P], F32)
        nc.vector.memset(ones_f[:], 1.0)
        ones_s = constp.tile([1, P], F32R)
        nc.vector.tensor_copy(ones_s[:], ones_f[:])
        vones_f = constp.tile([P, BT // P], F32)
        nc.vector.memset(vones_f[:], 1.0)

        wq_s = wpool.tile([P, KT, P], F32R, tag="wq")
        wk_s = wpool.tile([P, KT, P], F32R, tag="wk")
        wv_s = wpool.tile([P, KT, P], F32R, tag="wv")
        nc.sync.dma_start(out=wq_s[:], in_=wqkv[0].bitcast(F32R))
        nc.sync.dma_start(out=wk_s[:], in_=wqkv[1].bitcast(F32R))
        nc.sync.dma_start(out=wv_s[:], in_=wqkv[2].bitcast(F32R))

        # qT/kT for the pair: [128 (pair dims), BT]
        qT_s = qkpool.tile([P, BT], F32R, tag="qT")
        kT_s = qkpool.tile([P, BT], F32R, tag="kT")
        for w_s, dst, ev in (
            (wq_s, qT_s, lambda o, i: nc.scalar.copy(o, i)),
            (wk_s, kT_s, lambda o, i: nc.vector.tensor_copy(o, i)),
        ):
            for qi in range(BT // QS):
                pp = ps_big.tile([P, QS], F32, tag="pbig")
                for k in range(KT):
                    _mm(
                        nc, pp[:],
                        w_s[:, k, :],
                        xT_s[:, k, qi * QS:(qi + 1) * QS],
                        start=(k == 0),
                        stop=(k == KT - 1),
                    )
                ev(dst[:, qi * QS:(qi + 1) * QS], pp[:])

        # v for the pair, augmented with a ones column after each head:
        # [128 kv, 16 tiles, 130] ; cols 0..63 = head-0 dims, col 64 = ones,
        # cols 65..128 = head-1 dims, col 129 = ones. Each head's AV lhsT
        # window [65e : 65e+65] = [64 dims, ones] -> psum rows 0..63 = out
        # dims, row 64 = softmax denominator (aligned partition start).
        v_s = vpool.tile([P, BT // P, 130], F32R, tag="v")
        nc.vector.tensor_copy(v_s[:, :, 64:65], vones_f[:, :, None])
        nc.vector.tensor_copy(v_s[:, :, 129:130], vones_f[:, :, None])
        for bt in range(BT // P):
            pv = ps_big.tile([P, QS], F32, tag="pbig")
            for k in range(KT):
                _mm(
                    nc, pv[:, :P],
                    xT_s[:, k, bt * P:(bt + 1) * P],
                    wv_s[:, k, :],
                    start=(k == 0),
                    stop=(k == KT - 1),
                )
            if bt % 2:
                nc.scalar.copy(v_s[:, bt, 0:64], pv[:, 0:64])
                nc.scalar.copy(v_s[:, bt, 65:129], pv[:, 64:128])
            else:
                nc.vector.tensor_copy(v_s[:, bt, 0:64], pv[:, 0:64])
                nc.vector.tensor_copy(v_s[:, bt, 65:129], pv[:, 64:128])

        # attention per (batch, head-in-pair, q strip)
        for b in range(B):
            for e in range(2):
                prow = slice(e * HD, (e + 1) * HD)
                voff = 65 * e
                q0 = b * T
                for qi in range(T // QS):
                    nkv = 4 * qi + 4
                    exps = []
                    for nj in range(nkv):
                        sp = ps_big.tile([P, QS], F32, tag="pbig")
                        _mm(
                            nc, sp[:],
                            kT_s[prow, q0 + nj * P: q0 + (nj + 1) * P],
                            qT_s[prow, q0 + qi * QS: q0 + (qi + 1) * QS],
                            start=True,
                            stop=True,
                        )
                        ex = epool.tile([P, QS], F32R, tag="exp")
                        nc.scalar.activation(
                            ex[:], sp[:], mybir.ActivationFunctionType.Exp)
                        t = nj - 4 * qi
                        if t >= 0:
                            nc.vector.tensor_mul(ex[:], ex[:], mask_s[:, t, :])
                        exps.append(ex)

                    po = ps_o.tile([65, QS], F32, tag="po")
                    for nj in range(nkv):
                        _mm(
                            nc, po[:],
                            v_s[:, b * NKV + nj, voff:voff + 65],
                            exps[nj][:],
                            start=(nj == 0),
                            stop=(nj == nkv - 1),
                        )
                    # normalize along q (free dim): reciprocal of the denom
                    # row, broadcast to 64 partitions via a K=1 matmul
                    rec = opool.tile([1, QS], F32R, tag="rec")
                    with nc.allow_low_precision(
                            reason="fp32r rounding of softmax reciprocal"):
                        nc.vector.reciprocal(rec[:], po[64:65, :])
                    pb = ps_b.tile([64, QS], F32, tag="pb")
                    _mm(nc, pb[:], ones_s[:, :64], rec[:], start=True, stop=True)
                    rec_b = opool.tile([64, QS], F32, tag="recb")
                    nc.scalar.copy(rec_b[:], pb[:])
                    xo = opool.tile([64, QS], F32, tag="xo")
                    nc.vector.tensor_mul(xo[:], po[0:64, :], rec_b[:])
                    nc.sync.dma_start(
                        out=ag_in[e * HD:(e + 1) * HD,
                                  q0 + qi * QS: q0 + (qi + 1) * QS],
                        in_=xo[:],
                    )


def _lm_head_phase(nc, tc, wp, w_tiles, wlmT, ag_out, logitsT):
    """logits shard [VS, BT] = W_shard[C, VS].T @ x_outT[C, BT]."""
    from contextlib import ExitStack

    with ExitStack() as ctx:
        xop = ctx.enter_context(tc.tile_pool(name="xop", bufs=1))
        outp = ctx.enter_context(tc.tile_pool(name="outp", bufs=8))
        psp = ctx.enter_context(tc.tile_pool(name="psp", bufs=8, space="PSUM"))

        xout_s = xop.tile([P, KT, BT], F32R)
        # 16 half-row DMAs to spread the 8MB load over more queues
        for k in range(KT):
            nc.sync.dma_start(
                out=xout_s[:, k, :BT // 2], in_=ag_out[k, :, :BT // 2].bitcast(F32R))
            nc.sync.dma_start(
                out=xout_s[:, k, BT // 2:], in_=ag_out[k, :, BT // 2:].bitcast(F32R))

        for m in range(MT):
            if m < WPREF:
                w_s = w_tiles[m]
            else:
                w_s = wp.tile([P, KT, P], F32R, tag="w")
                nc.sync.dma_start(out=w_s[:], in_=wlmT[m].bitcast(F32R))
            for n in range(NT):
                ps = psp.tile([P, QS], F32, tag="ps")
                for k in range(KT):
                    _mm(
                        nc, ps[:],
                        w_s[:, k, :],
                        xout_s[:, k, n * QS:(n + 1) * QS],
                        start=(k == 0),
                        stop=(k == KT - 1),
                    )
                o_s = outp.tile([P, QS], F16, tag="o")
                if (m * NT + n) % 2:
                    nc.scalar.copy(o_s[:], ps[:])
                else:
                    nc.vector.tensor_copy(o_s[:], ps[:])
                nc.sync.dma_start(
                    out=logitsT[:, m, n * QS:(n + 1) * QS], in_=o_s[:])


def _host_prep(idx, tok_emb, pos_emb, Wq, Wk, Wv, W_lm):
    x = tok_emb[idx.reshape(-1)].astype(np.float32) + np.tile(
        pos_emb[:T].astype(np.float32), (B, 1)
    )  # [BT, C]
    xT_in = np.ascontiguousarray(
        x.T.reshape(KT, P, BT).transpose(1, 0, 2)
    )  # [P, KT, BT]

    NPAIR = H // 2

    def pack_w(W):
        # W [H, C, HD] -> [NPAIR, P, KT, 128] with [j,p,k,e*64+d] = W[2j+e, k*128+p, d]
        return np.ascontiguousarray(
            W.reshape(NPAIR, 2, KT, P, HD).transpose(0, 3, 2, 1, 4).reshape(
                NPAIR, P, KT, P
            )
        )

    wqkv = np.stack([
        pack_w(Wq.astype(np.float32) * (C ** -0.5)),
        pack_w(Wk.astype(np.float32)),
        pack_w(Wv.astype(np.float32)),
    ])  # [3, NPAIR, P, KT, P]

    pm = np.arange(P)[:, None]
    fm = np.arange(QS)[None, :]
    masks = np.stack(
        [(fm >= t * P + pm).astype(np.float32) for t in range(4)], axis=1
    )  # [P, 4, QS]

    W_lm_pad = np.zeros((VPAD, C), np.float32)
    W_lm_pad[:V] = W_lm.astype(np.float32)
    wlmT_shards = []
    for r in range(NCORES):
        sh = W_lm_pad[r * VS:(r + 1) * VS]  # [VS, C]
        # [MT, P, KT, P] with [m, p, k, j] = W[m*128 + j, k*128 + p]
        wlmT_shards.append(np.ascontiguousarray(
            sh.reshape(MT, P, KT, P).transpose(0, 3, 2, 1)
        ))
    return xT_in, wqkv, masks, wlmT_shards


def kernel(idx, tok_emb, pos_emb, Wq, Wk, Wv, W_lm, b_lm, _trace=False):
    idx = np.asarray(idx)
    xT_in, wqkv, masks, wlmT_shards = _host_prep(
        np.asarray(idx), np.asarray(tok_emb), np.asarray(pos_emb),
        np.asarray(Wq), np.asarray(Wk), np.asarray(Wv), np.asarray(W_lm),
    )
    nc = _build_program()
    in_maps = [
        {
            "xT": xT_in,
            "wqkv": np.ascontiguousarray(wqkv[:, r]),
            "wlmT": wlmT_shards[r],
            "masks": masks,
        }
        for r in range(NCORES)
    ]
    res = run_bass_kernel_spmd(nc, in_maps, list(range(NCORES)), trace=_trace)
    parts = []
    for r in range(NCORES):
        lt = res.results[r]["logitsT"]  # [P, MT, BT] fp16
        parts.append(
            np.asarray(lt).astype(np.float32).transpose(1, 0, 2).reshape(VS, BT))
    full = np.concatenate(parts, axis=0)[:V]          # [V, BT]
    logits = np.ascontiguousarray(full.T).reshape(B, T, V)
    b_lm = np.asarray(b_lm, dtype=np.float32)
    if np.any(b_lm):
        logits = logits + b_lm
    if _trace:
        kernel._last_exec_time_ns = res.exec_time_ns
        kernel._last_profile_json = res.profile_json
    return logits.astype(np.float32)
